# revision 1
# baseline (speedup 1.0000x reference)
"""Trainium2 Bass kernel for DihedralToCartesian (NeRF-style dihedral->xyz chain).

Full-input contract: kernel(angles[65536,252], prev_three[65536,3,3]) -> [65536,126,3].
Internally: batch is sharded 8 ways (8192 rows/core, pure data parallelism).

Math restructuring (validated vs the JAX reference, full batch, rel err ~3e-5):
the per-atom step
    bc = norm(b-c); n = norm((b-a) x bc); m1 = n x bc
    d  = c + r0*bc + r1*m1 + r2*n
is an affine chain over a frame F = [f1, f2, f3] = [bc, m1, n]:
    h    = cB*f2 - sB*f3          (cB,sB = eps-damped cos/sin of dihedral)
    p   += bond*cosA*f1 + bond*sinA*h
    f1'  = (-cosA*f1 - sinA*h) * invw
    f2'  = (sinA*g2*f1 - cosA*h) * invw*invg
    f3'  = (sB*f2 + cB*f3) * invg
where g2 = cB^2+sB^2 (slightly < 1 when sin^2+cos^2 is tiny, because the
reference adds 1e-8 inside its normalize), invg = rsqrt(g2) and
invw = rsqrt(cos^2 A + sin^2 A * g2).  These per-atom normalizers reproduce
exactly the frame tilt the reference gets from renormalizing eps-damped
vectors; without them, rare tiny-dihedral rows diverge to ~1e-2.

Per-core layout: batch row beta = 64*p + j (p = SBUF partition, j in [0,64)).
Recurrence ops are [128, 3, 64] fp32 (comp-planar).  All rsqrts are done as
exp(-0.5*ln(x)) on ScalarE (the Rsqrt table is banned as inaccurate).
Dihedral precompute is chunked by atom range and interleaved with the step
loop so it hides behind the recurrence.
"""

import os
import sys

import numpy as np

for _p in ("/opt/trn_rl_repo", os.path.expanduser("~/.axon_site/_ro/trn_rl_repo")):
    if os.path.isdir(_p) and _p not in sys.path:
        sys.path.insert(0, _p)

import concourse.bass as bass
import concourse.bacc as bacc
import concourse.mybir as mybir
import concourse.tile as tile
from concourse.bass_utils import run_bass_kernel_spmd

F32 = mybir.dt.float32
AOP = mybir.AluOpType
AF = mybir.ActivationFunctionType

N_CORES = 8
B_FULL = 65536
BS = B_FULL // N_CORES  # 8192 rows per core
N = 126                 # atoms
P = 128                 # partitions
J = BS // P             # 64 batch columns per partition
BLK = 18                # atoms per output staging block
CH_A = 6                # atoms per precompute chunk

_ALPHA = np.array([2.028, 2.124, 1.941], np.float32)
_BOND = np.array([1.329, 1.458, 1.523], np.float32)
_CA = np.cos(_ALPHA)
_SA = np.sin(_ALPHA)
_C2A = (_CA * _CA).astype(np.float32)
_S2A = (np.float32(1.0) - _C2A).astype(np.float32)
_BCA = _BOND * _CA
_BSA = _BOND * _SA


def _emit(nc: bass.Bass):
    angles = nc.dram_tensor("angles", [BS, 2 * N], F32, kind="ExternalInput").ap()
    prev = nc.dram_tensor("prev_three", [BS, 3, 3], F32, kind="ExternalInput").ap()
    out = nc.dram_tensor("out", [BS, N, 3], F32, kind="ExternalOutput").ap()

    ang_r = angles.rearrange("(p j) c -> p j c", p=P)          # [128, 64, 252]
    prev_r = prev.rearrange("(p j) r c -> p j (r c)", p=P)     # [128, 64, 9]
    out_r = out.rearrange("(p j) a c -> p j (a c)", p=P)       # [128, 64, 378]

    with tile.TileContext(nc) as tc:
        with (
            tc.tile_pool(name="planes", bufs=1) as planes,
            tc.tile_pool(name="stag", bufs=2) as stagp,
            tc.tile_pool(name="chunk", bufs=1) as chunk,
            tc.tile_pool(name="state", bufs=2) as state,
            tc.tile_pool(name="scratch", bufs=2) as scratch,
        ):
            # persistent planes, f = 126*j + a
            rawS = planes.tile([P, J * N], F32, tag="rawS")  # raw sin -> invg
            rawC = planes.tile([P, J * N], F32, tag="rawC")  # raw cos -> invw
            cdP = planes.tile([P, J * N], F32, tag="cdP")    # damped cos(theta)
            sdP = planes.tile([P, J * N], F32, tag="sdP")    # damped sin(theta)
            pv = planes.tile([P, J * 9], F32, tag="pv")
            c2aT = planes.tile([P, N], F32, tag="c2aT")      # cos^2(alpha) pattern
            s2aT = planes.tile([P, N], F32, tag="s2aT")      # sin^2(alpha) pattern

            nc.sync.dma_start(
                out=rawS[:].rearrange("p (j a) -> p j a", a=N), in_=ang_r[:, :, 0:N]
            )
            nc.sync.dma_start(
                out=rawC[:].rearrange("p (j a) -> p j a", a=N),
                in_=ang_r[:, :, N : 2 * N],
            )
            nc.sync.dma_start(
                out=pv[:].rearrange("p (j x) -> p j x", x=9), in_=prev_r
            )
            for k in range(3):
                v3 = c2aT[:].rearrange("p (a k) -> p a k", k=3)[:, :, k]
                nc.vector.memset(v3, float(_C2A[k]))
                v3 = s2aT[:].rearrange("p (a k) -> p a k", k=3)[:, :, k]
                nc.vector.memset(v3, float(_S2A[k]))

            # atom-major views [128, a, j] / chunk views [128, j, a]
            def aview(t):
                return t[:].rearrange("p (j a) -> p a j", a=N)

            def jview(t):
                return t[:].rearrange("p (j a) -> p j a", a=N)

            # ---- initial frame from prev_three -------------------------------
            pv_r = pv[:].rearrange("p (j x) -> p x j", x=9)      # [128, 9, 64]
            a_ap = pv_r[:, 0:3, :]
            b_ap = pv_r[:, 3:6, :]
            c_ap = pv_r[:, 6:9, :]

            def cross(dst, x, y, eps):
                for c in range(3):
                    c1, c2 = (c + 1) % 3, (c + 2) % 3
                    m = scratch.tile([P, 1, J], F32, tag="cr_m")
                    qt = scratch.tile([P, 1, J], F32, tag="cr_q")
                    nc.vector.tensor_mul(m[:], x[:, c1 : c1 + 1, :], y[:, c2 : c2 + 1, :])
                    nc.vector.tensor_mul(qt[:], x[:, c2 : c2 + 1, :], y[:, c1 : c1 + 1, :])
                    nc.vector.scalar_tensor_tensor(
                        dst[:, c : c + 1, :], m[:], eps, qt[:], AOP.add, AOP.subtract
                    )

            def rsqrt3(dst, src3):
                sq = scratch.tile([P, 3, J], F32, tag="in_sq")
                nc.scalar.square(sq[:], src3[:])
                s1 = scratch.tile([P, J], F32, tag="in_s1")
                nc.vector.tensor_add(s1[:], sq[:, 0, :], sq[:, 1, :])
                s2_ = scratch.tile([P, J], F32, tag="in_s2")
                nc.vector.tensor_add(s2_[:], s1[:], sq[:, 2, :])
                lgi = scratch.tile([P, J], F32, tag="in_lg")
                nc.scalar.activation(lgi[:], s2_[:], AF.Ln)
                nc.scalar.activation(dst[:], lgi[:], AF.Exp, 0.0, -0.5)

            vv = scratch.tile([P, 3, J], F32, tag="in_v")
            nc.vector.scalar_tensor_tensor(
                vv[:], b_ap, 1e-8, c_ap, AOP.add, AOP.subtract
            )
            rv1 = scratch.tile([P, J], F32, tag="in_rv")
            rsqrt3(rv1, vv)
            f1 = state.tile([P, 3, J], F32, tag="f1")
            nc.vector.tensor_mul(
                f1[:], vv[:], rv1[:].unsqueeze(1).broadcast_to([P, 3, J])
            )
            uu = scratch.tile([P, 3, J], F32, tag="in_u")
            nc.vector.tensor_sub(uu[:], b_ap, a_ap)
            ww = scratch.tile([P, 3, J], F32, tag="in_w")
            cross(ww, uu, f1, 1e-8)
            rw = scratch.tile([P, J], F32, tag="in_rw")
            rsqrt3(rw, ww)
            f3 = state.tile([P, 3, J], F32, tag="f3")
            nc.vector.tensor_mul(
                f3[:], ww[:], rw[:].unsqueeze(1).broadcast_to([P, 3, J])
            )
            f2 = state.tile([P, 3, J], F32, tag="f2")
            cross(f2, f3, f1, 0.0)

            # ---- fused: precompute chunks interleaved with the chain ---------
            p_prev_ap = c_ap
            stag_tiles = [None, None]

            def emit_chunk(k):
                asl = slice(CH_A * k, CH_A * (k + 1))
                rS = jview(rawS)[:, :, asl]
                rC = jview(rawC)[:, :, asl]
                cD = jview(cdP)[:, :, asl]
                sD = jview(sdP)[:, :, asl]
                SH = [P, J, CH_A]
                s2 = chunk.tile(SH, F32, tag="s2")
                nc.scalar.square(s2[:], rS)
                c2 = chunk.tile(SH, F32, tag="c2")
                nc.scalar.square(c2[:], rC)
                ss = chunk.tile(SH, F32, tag="ss")
                nc.vector.scalar_tensor_tensor(
                    ss[:], s2[:], 1e-8, c2[:], AOP.add, AOP.add
                )
                lg = chunk.tile(SH, F32, tag="lg")
                nc.scalar.activation(lg[:], ss[:], AF.Ln)
                rv = chunk.tile(SH, F32, tag="rv")
                nc.scalar.activation(rv[:], lg[:], AF.Exp, 0.0, -0.5)
                nc.vector.tensor_mul(cD, rC, rv[:])
                nc.vector.tensor_mul(sD, rS, rv[:])
                gc = chunk.tile(SH, F32, tag="s2", name=f"gc{k}")
                nc.scalar.square(gc[:], cD)
                gs = chunk.tile(SH, F32, tag="c2", name=f"gs{k}")
                nc.scalar.square(gs[:], sD)
                gg = chunk.tile(SH, F32, tag="gg")
                nc.gpsimd.tensor_add(gg[:], gc[:], gs[:])
                lg2 = chunk.tile(SH, F32, tag="lg", name=f"lg2_{k}")
                nc.scalar.activation(lg2[:], gg[:], AF.Ln)
                nc.scalar.activation(rS, lg2[:], AF.Exp, 0.0, -0.5)  # invg -> rawS
                c2a_b = (
                    c2aT[:, asl].unsqueeze(1).broadcast_to([P, J, CH_A])
                )
                s2a_b = (
                    s2aT[:, asl].unsqueeze(1).broadcast_to([P, J, CH_A])
                )
                mw = chunk.tile(SH, F32, tag="mw")
                nc.vector.tensor_mul(mw[:], gg[:], s2a_b)
                w2 = chunk.tile(SH, F32, tag="w2")
                nc.vector.tensor_add(w2[:], mw[:], c2a_b)
                lg3 = chunk.tile(SH, F32, tag="lg", name=f"lg3_{k}")
                nc.scalar.activation(lg3[:], w2[:], AF.Ln)
                nc.scalar.activation(rC, lg3[:], AF.Exp, 0.0, -0.5)  # invw -> rawC

            cdA, sdA, igA, iwA = aview(cdP), aview(sdP), aview(rawS), aview(rawC)

            for i in range(N):
                if i % CH_A == 0:
                    emit_chunk(i // CH_A)
                k3 = i % 3
                ca, sa = float(_CA[k3]), float(_SA[k3])
                bca, bsa = float(_BCA[k3]), float(_BSA[k3])
                blk, al = i // BLK, i % BLK
                last = i == N - 1
                if al == 0:
                    stag_tiles[blk % 2] = stagp.tile(
                        [P, J * 3 * BLK], F32, tag="stag", name=f"stag{blk}"
                    )
                stag = stag_tiles[blk % 2]
                stag_r = stag[:].rearrange("p (j x) -> p x j", x=3 * BLK)

                cb1 = cdA[:, i : i + 1, :]
                sb1 = sdA[:, i : i + 1, :]
                ig1 = igA[:, i : i + 1, :]
                iw1 = iwA[:, i : i + 1, :]
                cb = cb1.broadcast_to([P, 3, J])
                sb = sb1.broadcast_to([P, 3, J])
                ig_b = ig1.broadcast_to([P, 3, J])
                iw_b = iw1.broadcast_to([P, 3, J])

                if not last:
                    # per-atom g^2 and invw*invg (small [128,1,64] ops)
                    sqc = scratch.tile([P, 1, J], F32, tag="sqc")
                    nc.scalar.square(sqc[:], cb1)
                    sqs = scratch.tile([P, 1, J], F32, tag="sqs")
                    nc.scalar.square(sqs[:], sb1)
                    ggs = scratch.tile([P, 1, J], F32, tag="ggs")
                    nc.vector.tensor_add(ggs[:], sqc[:], sqs[:])
                    iwg = scratch.tile([P, 1, J], F32, tag="iwg")
                    nc.vector.tensor_mul(iwg[:], iw1, ig1)
                    # early ACT/DVE work off the critical h-chain
                    fc = scratch.tile([P, 3, J], F32, tag="fc")
                    nc.scalar.mul(fc[:], f1[:], ca)
                    t9 = scratch.tile([P, 3, J], F32, tag="t9")
                    nc.vector.tensor_mul(
                        t9[:], f1[:], ggs[:].broadcast_to([P, 3, J])
                    )
                    t9s = scratch.tile([P, 3, J], F32, tag="t9s")
                    nc.scalar.mul(t9s[:], t9[:], sa)

                tmp = scratch.tile([P, 3, J], F32, tag="tmp")
                nc.vector.scalar_tensor_tensor(
                    tmp[:], f1[:], bca, p_prev_ap, AOP.mult, AOP.add
                )
                t1 = scratch.tile([P, 3, J], F32, tag="t1")
                nc.vector.tensor_mul(t1[:], f2[:], cb)
                t2 = scratch.tile([P, 3, J], F32, tag="t2")
                nc.vector.tensor_mul(t2[:], f3[:], sb)
                h = scratch.tile([P, 3, J], F32, tag="h")
                nc.vector.tensor_sub(h[:], t1[:], t2[:])

                pn_ap = stag_r[:, 3 * al : 3 * al + 3, :]
                nc.vector.scalar_tensor_tensor(
                    pn_ap, h[:], bsa, tmp[:], AOP.mult, AOP.add
                )

                if not last:
                    f1p = scratch.tile([P, 3, J], F32, tag="f1p")
                    nc.vector.scalar_tensor_tensor(
                        f1p[:], h[:], -sa, fc[:], AOP.mult, AOP.subtract
                    )
                    f1n = state.tile([P, 3, J], F32, tag="f1")
                    nc.vector.tensor_mul(f1n[:], f1p[:], iw_b)
                    f2p = scratch.tile([P, 3, J], F32, tag="f2p")
                    nc.vector.scalar_tensor_tensor(
                        f2p[:], h[:], -ca, t9s[:], AOP.mult, AOP.add
                    )
                    f2n = state.tile([P, 3, J], F32, tag="f2")
                    nc.vector.tensor_mul(
                        f2n[:], f2p[:], iwg[:].broadcast_to([P, 3, J])
                    )
                    t3 = scratch.tile([P, 3, J], F32, tag="t3")
                    nc.gpsimd.tensor_mul(t3[:], f2[:], sb)
                    t4 = scratch.tile([P, 3, J], F32, tag="t4")
                    nc.gpsimd.tensor_mul(t4[:], f3[:], cb)
                    f3s = scratch.tile([P, 3, J], F32, tag="f3s")
                    nc.gpsimd.tensor_add(f3s[:], t3[:], t4[:])
                    f3n = state.tile([P, 3, J], F32, tag="f3")
                    nc.gpsimd.tensor_mul(f3n[:], f3s[:], ig_b)
                    f1, f2, f3 = f1n, f2n, f3n
                p_prev_ap = pn_ap

                if al == BLK - 1:
                    nc.sync.dma_start(
                        out=out_r[:, :, 3 * BLK * blk : 3 * BLK * (blk + 1)],
                        in_=stag[:].rearrange("p (j x) -> p j x", x=3 * BLK),
                    )
    return nc


_NC_CACHE: dict = {}


def _get_nc():
    if "nc" not in _NC_CACHE:
        nc = bacc.Bacc("TRN2", target_bir_lowering=False, debug=False)
        _emit(nc)
        nc.compile()
        _NC_CACHE["nc"] = nc
    return _NC_CACHE["nc"]


def run_sharded(angles: np.ndarray, prev_three: np.ndarray, **kw):
    """Shard inputs over 8 cores, run, return BassKernelResults."""
    angles = np.ascontiguousarray(angles, np.float32)
    prev_three = np.ascontiguousarray(prev_three, np.float32)
    assert angles.shape == (B_FULL, 2 * N) and prev_three.shape == (B_FULL, 3, 3)
    in_maps = [
        {
            "angles": angles[i * BS : (i + 1) * BS],
            "prev_three": prev_three[i * BS : (i + 1) * BS],
        }
        for i in range(N_CORES)
    ]
    return run_bass_kernel_spmd(_get_nc(), in_maps, core_ids=list(range(N_CORES)), **kw)


def kernel(angles: np.ndarray, prev_three: np.ndarray) -> np.ndarray:
    res = run_sharded(angles, prev_three)
    return np.concatenate([r["out"] for r in res.results], axis=0)



# revision 6
# speedup vs baseline: 1.8931x; 1.8931x over previous
"""Trainium2 Bass kernel for DihedralToCartesian (NeRF-style dihedral->xyz chain).

Full-input contract: kernel(angles[65536,252], prev_three[65536,3,3]) -> [65536,126,3].
Batch sharded 8 ways (8192 rows/core, [128 partitions x 64 cols], pure data
parallelism).

Math: the reference renormalizes its frame every step, so the frame follows
the PURE-UNIT dihedral direction (cb,sb) = (c,s)/sqrt(s^2+c^2) exactly; the
damping eps only shrinks the per-step bond displacement by g = |damped| which
is 1 to within 5e-7 for all but ~4 rows of the dataset (worst-case once-off
~1e-3 rel).  So we use (cd,sd) = (c,s)*rsqrt(s^2+c^2+1e-12) and treat the
step as an exact rotation -- no invw/invg normalizers at all:

    h  = cd*f2 - sd*f3
    u  = ca*f1 + sa*h          p' = p + bond*u
    f1' = -u                   f2' = sa*f1 - ca*h      f3' = sd*f2 + cd*f3

Sign-folding removes the f1' negation: track g1 = (-1)^(i+1) f1_i,
g2h = (-1)^i f2_i, g3h = (-1)^i f3_i; then with hh = cd*g2h - sd*g3h:
    g1' = ca*g1 - sa*hh        g2h' = sa*g1 + ca*hh
    g3h' = -(sd*g2h + cd*g3h)  p'  = p + (-1)^i bond * g1'
All per-atom constants (ca, sa, bond, sign) are compile-time.

Layout: states [128, 3, 64] fp32; dihedral planes stored ATOM-MAJOR so the
per-atom slices are contiguous (the strided-read tax is paid once in the
precompute transposes, not 4x per atom).  Precompute is chunked (CH atoms)
and paced one sub-op per atom so the per-engine program order never blocks
the recurrence; activation phases batch to ~3 table loads total.
"""

import json
import os
import sys

import numpy as np

for _p in ("/opt/trn_rl_repo", os.path.expanduser("~/.axon_site/_ro/trn_rl_repo")):
    if os.path.isdir(_p) and _p not in sys.path:
        sys.path.insert(0, _p)

import concourse.bass as bass
import concourse.bacc as bacc
import concourse.mybir as mybir
import concourse.tile as tile
from concourse.bass_utils import run_bass_kernel_spmd

F32 = mybir.dt.float32
AOP = mybir.AluOpType
AF = mybir.ActivationFunctionType

N_CORES = 8
B_FULL = 65536
BS = B_FULL // N_CORES
N = 126
P = 128
J = BS // P            # 64
BLK = 21               # atoms per output staging block (6 blocks)
CH = 9                 # atoms per precompute chunk (14 chunks)
NCH = N // CH

_ALPHA = np.array([2.028, 2.124, 1.941], np.float64)
_BOND = np.array([1.329, 1.458, 1.523], np.float64)
_CA = np.cos(_ALPHA)
_SA = np.sin(_ALPHA)

# engine assignment, overridable: KERN_ENG='{"t3":"v",...}'
# NOTE: scalar_tensor_tensor does NOT compile on Pool ("g") in this env --
# keep all STT ops (g1n/g2n/g3n/pn/ss) on "v".
ENG = {
    "t1": "v", "t2": "v", "hh": "v", "g1n": "v", "g2n": "v", "g3n": "v",
    "t4": "v", "t3": "g", "pn": "v",
    "fc": "s", "fs": "s",
    "pc0": "s", "pc1": "g",  # staging copy engine, even/odd atoms
    # chunk sub-ops
    "sT": "s", "cT": "s", "s2": "s", "c2": "s", "lg": "s", "rv": "s",
    "ss": "v", "cdm": "g", "sdm": "g",
}
ENG.update(json.loads(os.environ.get("KERN_ENG", "{}")))
PLANES_F16 = os.environ.get("PLANES_F16", "0") == "1"
PF = mybir.dt.float16 if PLANES_F16 else F32


def _emit(nc: bass.Bass):
    angles = nc.dram_tensor("angles", [BS, 2 * N], F32, kind="ExternalInput").ap()
    prev = nc.dram_tensor("prev_three", [BS, 3, 3], F32, kind="ExternalInput").ap()
    out = nc.dram_tensor("out", [BS, N, 3], F32, kind="ExternalOutput").ap()

    ang_flat = angles.rearrange("(p j) c -> p (j c)", p=P)      # [128, 16128]
    prev_flat = prev.rearrange("(p j) r c -> p (j r c)", p=P)   # [128, 576]
    out_r = out.rearrange("(p j) a c -> p j (a c)", p=P)        # [128, 64, 378]

    def eng(key):
        return {"v": nc.vector, "g": nc.gpsimd, "s": nc.scalar}[ENG[key]]

    with tile.TileContext(nc) as tc:
        with (
            tc.tile_pool(name="raw", bufs=1) as rawp,
            tc.tile_pool(name="planes", bufs=1) as planesp,
            tc.tile_pool(name="chunk", bufs=1) as chunkp,
            tc.tile_pool(name="state", bufs=2) as statep,
            tc.tile_pool(name="scratch", bufs=2) as scratch,
            tc.tile_pool(name="stag", bufs=2) as stagp,
            tc.tile_pool(name="pv", bufs=1) as pvp,
        ):
            raw = rawp.tile([P, J * 2 * N], F32, tag="raw")      # 64.5KB j-major
            cdp = planesp.tile([P, N * J], PF, tag="cdp")        # atom-major
            sdp = planesp.tile([P, N * J], PF, tag="sdp")
            pv = pvp.tile([P, J * 9], F32, tag="pv")
            pvt = pvp.tile([P, 9 * J], F32, tag="pvt")           # [9, 64]

            nc.sync.dma_start(out=pv[:], in_=prev_flat)
            nc.sync.dma_start(out=raw[:], in_=ang_flat)

            raw_aj = raw[:].rearrange("p (j a) -> p a j", a=2 * N)  # strided view
            cd_a = cdp[:].rearrange("p (a j) -> p a j", j=J)
            sd_a = sdp[:].rearrange("p (a j) -> p a j", j=J)

            # ---------- precompute chunk sub-op emitters -----------------
            def chunk_ops(k):
                """Return list of thunks computing planes for atoms [CH*k, CH*k+CH)."""
                a0 = CH * k
                sl = slice(a0, a0 + CH)
                SH = [P, CH, J]
                st = chunkp.tile(SH, F32, tag="sT", name=f"sT{k}")
                ct = chunkp.tile(SH, F32, tag="cT", name=f"cT{k}")
                s2 = chunkp.tile(SH, F32, tag="s2", name=f"s2{k}")
                c2 = chunkp.tile(SH, F32, tag="c2", name=f"c2{k}")
                ssq = chunkp.tile(SH, F32, tag="ss", name=f"ss{k}")
                lg = chunkp.tile(SH, F32, tag="s2", name=f"lg{k}")
                rv = chunkp.tile(SH, F32, tag="c2", name=f"rv{k}")
                return [
                    lambda: eng("sT").copy(st[:], raw_aj[:, sl, :]),
                    lambda: eng("cT").copy(ct[:], raw_aj[:, N + a0 : N + a0 + CH, :]),
                    lambda: eng("s2").square(s2[:], st[:]),
                    lambda: eng("c2").square(c2[:], ct[:]),
                    lambda: eng("ss").scalar_tensor_tensor(
                        ssq[:], s2[:], 1e-12, c2[:], AOP.add, AOP.add),
                    lambda: eng("lg").activation(lg[:], ssq[:], AF.Ln),
                    lambda: eng("rv").activation(rv[:], lg[:], AF.Exp, 0.0, -0.5),
                    lambda: eng("cdm").tensor_mul(cd_a[:, sl, :], ct[:], rv[:]),
                    lambda: eng("sdm").tensor_mul(sd_a[:, sl, :], st[:], rv[:]),
                ]

            # ---------- initial frame from prev_three --------------------
            # transpose pv [j,9] -> pvt [9,j]
            nc.scalar.copy(
                pvt[:].rearrange("p (x j) -> p x j", x=9),
                pv[:].rearrange("p (j x) -> p x j", x=9),
            )
            pvt_r = pvt[:].rearrange("p (x j) -> p x j", x=9)
            a0_ap = pvt_r[:, 0:3, :]
            b0_ap = pvt_r[:, 3:6, :]
            c0_ap = pvt_r[:, 6:9, :]

            def rsqrt3(dst, src3, tag):
                sq = scratch.tile([P, 3, J], F32, tag="i_sq", name=f"sq_{tag}")
                nc.scalar.square(sq[:], src3[:])
                s1 = scratch.tile([P, 1, J], F32, tag="i_s1", name=f"s1_{tag}")
                nc.vector.tensor_add(s1[:], sq[:, 0:1, :], sq[:, 1:2, :])
                s2_ = scratch.tile([P, 1, J], F32, tag="i_s2", name=f"s2_{tag}")
                nc.vector.tensor_add(s2_[:], s1[:], sq[:, 2:3, :])
                lgi = scratch.tile([P, 1, J], F32, tag="i_lg", name=f"lg_{tag}")
                nc.scalar.activation(lgi[:], s2_[:], AF.Ln)
                nc.scalar.activation(dst[:], lgi[:], AF.Exp, 0.0, -0.5)

            vv = scratch.tile([P, 3, J], F32, tag="i_vv")
            nc.vector.scalar_tensor_tensor(
                vv[:], b0_ap, 1e-8, c0_ap, AOP.add, AOP.subtract)
            rv1 = scratch.tile([P, 1, J], F32, tag="i_rv")
            rsqrt3(rv1, vv, "f1")
            g1 = statep.tile([P, 3, J], F32, tag="g1", name="g1_init")
            nc.vector.tensor_mul(g1[:], vv[:], rv1[:].broadcast_to([P, 3, J]))

            uu = scratch.tile([P, 3, J], F32, tag="i_uu")
            nc.vector.tensor_sub(uu[:], b0_ap, a0_ap)
            # g3h_init = -f3 = normalize(cross(f1, b-a) - 1e-8)
            ww = scratch.tile([P, 3, J], F32, tag="i_ww")
            for c in range(3):
                c1, c2 = (c + 1) % 3, (c + 2) % 3
                m = scratch.tile([P, 1, J], F32, tag="i_cm", name=f"cw_m{c}")
                q = scratch.tile([P, 1, J], F32, tag="i_cq", name=f"cw_q{c}")
                nc.vector.tensor_mul(m[:], g1[:, c1 : c1 + 1, :], uu[:, c2 : c2 + 1, :])
                nc.vector.tensor_mul(q[:], g1[:, c2 : c2 + 1, :], uu[:, c1 : c1 + 1, :])
                nc.vector.scalar_tensor_tensor(
                    ww[:, c : c + 1, :], m[:], -1e-8, q[:], AOP.add, AOP.subtract)
            rv2 = scratch.tile([P, 1, J], F32, tag="i_rv", name="i_rv2")
            rsqrt3(rv2, ww, "f3")
            g3h = statep.tile([P, 3, J], F32, tag="g3", name="g3_init")
            nc.vector.tensor_mul(g3h[:], ww[:], rv2[:].broadcast_to([P, 3, J]))
            # g2h_init = -f2 = cross(g3h_init, f1)
            g2h = statep.tile([P, 3, J], F32, tag="g2", name="g2_init")
            for c in range(3):
                c1, c2 = (c + 1) % 3, (c + 2) % 3
                m = scratch.tile([P, 1, J], F32, tag="i_cm", name=f"c2m{c}")
                q = scratch.tile([P, 1, J], F32, tag="i_cq", name=f"c2q{c}")
                nc.vector.tensor_mul(m[:], g3h[:, c1 : c1 + 1, :], g1[:, c2 : c2 + 1, :])
                nc.vector.tensor_mul(q[:], g3h[:, c2 : c2 + 1, :], g1[:, c1 : c1 + 1, :])
                nc.vector.tensor_sub(g2h[:, c : c + 1, :], m[:], q[:])

            # chunks 0 and 1 fully before the loop (one chunk of slack)
            for f in chunk_ops(0):
                f()
            for f in chunk_ops(1):
                f()
            pending: list = list(chunk_ops(2))
            next_chunk = 3

            # ---------- main recurrence ---------------------------------
            p_prev_ap = c0_ap
            stag_tiles = [None, None]
            stag_views = [None, None]
            for i in range(N):
                k3 = i % 3
                ca, sa = float(_CA[k3]), float(_SA[k3])
                sbond = float(_BOND[k3] * (1.0 if i % 2 == 0 else -1.0))
                blk, al = i // BLK, i % BLK
                if al == 0:
                    stag_tiles[blk % 2] = stagp.tile(
                        [P, J * 3 * BLK], F32, tag="stag", name=f"stag{blk}")
                    stag_views[blk % 2] = stag_tiles[blk % 2][:].rearrange(
                        "p (j x) -> p x j", x=3 * BLK)
                stag = stag_tiles[blk % 2]
                stag_r = stag_views[blk % 2]

                cdb = cd_a[:, i : i + 1, :].broadcast_to([P, 3, J])
                sdb = sd_a[:, i : i + 1, :].broadcast_to([P, 3, J])

                fc = scratch.tile([P, 3, J], F32, tag="fc", name=f"fc{i}")
                eng("fc").mul(fc[:], g1[:], ca)
                fs = scratch.tile([P, 3, J], F32, tag="fs", name=f"fs{i}")
                eng("fs").mul(fs[:], g1[:], sa)

                t3 = scratch.tile([P, 3, J], F32, tag="t3", name=f"t3_{i}")
                eng("t3").tensor_mul(t3[:], g2h[:], sdb)
                t1 = scratch.tile([P, 3, J], F32, tag="t1", name=f"t1_{i}")
                eng("t1").tensor_mul(t1[:], g2h[:], cdb)
                t2 = scratch.tile([P, 3, J], F32, tag="t2", name=f"t2_{i}")
                eng("t2").tensor_mul(t2[:], g3h[:], sdb)
                t4 = scratch.tile([P, 3, J], F32, tag="t4", name=f"t4_{i}")
                eng("t4").tensor_mul(t4[:], g3h[:], cdb)
                hh = scratch.tile([P, 3, J], F32, tag="hh", name=f"hh{i}")
                eng("hh").tensor_sub(hh[:], t1[:], t2[:])

                g1n = statep.tile([P, 3, J], F32, tag="g1", name=f"g1_{i}")
                eng("g1n").scalar_tensor_tensor(
                    g1n[:], hh[:], -sa, fc[:], AOP.mult, AOP.add)
                g2n = statep.tile([P, 3, J], F32, tag="g2", name=f"g2_{i}")
                eng("g2n").scalar_tensor_tensor(
                    g2n[:], hh[:], ca, fs[:], AOP.mult, AOP.add)
                g3n = statep.tile([P, 3, J], F32, tag="g3", name=f"g3_{i}")
                eng("g3n").scalar_tensor_tensor(
                    g3n[:], t3[:], -1.0, t4[:], AOP.mult, AOP.subtract)

                pd = statep.tile([P, 3, J], F32, tag="pd", name=f"pd{i}")
                eng("pn").scalar_tensor_tensor(
                    pd[:], g1n[:], sbond, p_prev_ap, AOP.mult, AOP.add)
                # copy dense p into j-major staging (strided write, off chain)
                pc_ap = stag_r[:, 3 * al : 3 * al + 3, :]
                pc_eng = eng("pc0") if i % 2 == 0 else eng("pc1")
                if pc_eng is nc.scalar:
                    pc_eng.copy(pc_ap, pd[:])
                else:
                    pc_eng.tensor_copy(pc_ap, pd[:])
                p_prev_ap = pd[:]
                g1, g2h, g3h = g1n, g2n, g3n

                # pace one precompute sub-op per atom
                if pending:
                    pending.pop(0)()
                elif next_chunk < NCH:
                    pending = list(chunk_ops(next_chunk))
                    next_chunk += 1
                    pending.pop(0)()

                if al == BLK - 1:
                    nc.sync.dma_start(
                        out=out_r[:, :, 3 * BLK * blk : 3 * BLK * (blk + 1)],
                        in_=stag[:].rearrange("p (j x) -> p j x", x=3 * BLK),
                    )
    return nc


_NC_CACHE: dict = {}


def _get_nc():
    if "nc" not in _NC_CACHE:
        nc = bacc.Bacc("TRN2", target_bir_lowering=False, debug=False)
        _emit(nc)
        nc.compile()
        _NC_CACHE["nc"] = nc
    return _NC_CACHE["nc"]


def run_sharded(angles: np.ndarray, prev_three: np.ndarray, **kw):
    angles = np.ascontiguousarray(angles, np.float32)
    prev_three = np.ascontiguousarray(prev_three, np.float32)
    assert angles.shape == (B_FULL, 2 * N) and prev_three.shape == (B_FULL, 3, 3)
    in_maps = [
        {
            "angles": angles[i * BS : (i + 1) * BS],
            "prev_three": prev_three[i * BS : (i + 1) * BS],
        }
        for i in range(N_CORES)
    ]
    return run_bass_kernel_spmd(_get_nc(), in_maps, core_ids=list(range(N_CORES)), **kw)


def kernel(angles: np.ndarray, prev_three: np.ndarray) -> np.ndarray:
    res = run_sharded(angles, prev_three)
    return np.concatenate([r["out"] for r in res.results], axis=0)


# revision 18
# speedup vs baseline: 2.2734x; 1.2009x over previous
"""Trainium2 Bass kernel for DihedralToCartesian (NeRF-style dihedral->xyz chain).

Full-input contract: kernel(angles[65536,252], prev_three[65536,3,3]) -> [65536,126,3].
Batch sharded 8 ways (8192 rows/core = [128 partitions x 64 cols], pure data
parallelism).

Math: the reference renormalizes its frame every step, so the frame follows the
PURE-UNIT dihedral direction (c,s)/sqrt(s^2+c^2) exactly; the 1e-8 damping only
shrinks one step's displacement by |damped| (~5e-7 off except ~4 dataset rows,
worst once-off ~1e-3 rel).  We therefore treat each step as an exact rotation
(no invw/invg normalizer chain):

    h = cd*f2 - sd*f3; u = ca*f1 + sa*h; p += bond*u
    f1' = -u; f2' = sa*f1 - ca*h; f3' = sd*f2 + cd*f3

Sign-folded states g1 = (-1)^(i+1) f1, g2h = (-1)^i f2, m3 = -(-1)^i f3 make
every step sign-free except the bond constant:
    hh  = cd*g2h + sd*m3         g1' = ca*g1 - sa*hh     g2h' = sa*g1 + ca*hh
    m3' = sd*g2h - cd*m3         p' = p + (+/-bond)*g1'
which is 6 plain TT multiplies/adds + 3 STT + 2 ACT const-scales per atom.

Everything runs in fp16 (validated vs the fp32 reference: rel err 9.5e-3 vs the
2e-2 gate): DVE gets its 2x 16-bit mode, SBUF port pressure and DMA bytes halve.
The rsqrt normalization chain stays per-op-rounded fp16 but with fp32-safe
structure.  Planes are stored ATOM-MAJOR so all per-atom loop operands are
contiguous.  Precompute is paced one sub-op per 2 atoms in chunk PAIRS with
activation phases batched (Ln,Ln,Exp,Exp) so table loads stay ~8 total.
Positions accumulate into a dense atom-major pblock; paced transpose pieces
copy it to a j-major staging buffer which DMAs out in ONE 128-descriptor
transfer (64 contiguous DRAM rows per partition).
"""

import json
import os
import sys

import numpy as np

for _p in ("/opt/trn_rl_repo", os.path.expanduser("~/.axon_site/_ro/trn_rl_repo")):
    if os.path.isdir(_p) and _p not in sys.path:
        sys.path.insert(0, _p)

import concourse.bass as bass
import concourse.bacc as bacc
import concourse.mybir as mybir
import concourse.tile as tile
from concourse.bass_utils import run_bass_kernel_spmd

F32 = mybir.dt.float32
F16 = mybir.dt.float16
AOP = mybir.AluOpType
AF = mybir.ActivationFunctionType

N_CORES = 8
B_FULL = 65536
BS = B_FULL // N_CORES
N = 126
P = 128
J = BS // P            # 64
CH = 18                # atoms per precompute chunk
GRP = 2                # chunks per pacing group (phase-batched)
TP = 6                 # atoms per transpose piece (N % TP == 0)

_ALPHA = np.array([2.028, 2.124, 1.941], np.float64)
_BOND = np.array([1.329, 1.458, 1.523], np.float64)
_CA = np.cos(_ALPHA)
_SA = np.sin(_ALPHA)

# engine assignment, overridable: KERN_ENG='{"t3":"v",...}'
# NOTE: scalar_tensor_tensor does NOT compile on Pool ("g") here -- STT ops
# (g1n/g2n/pn) must stay on "v".
ENG = {
    "t1": "v", "t2": "v", "hh": "v", "g1n": "v", "g2n": "v", "m3n": "v",
    "t4": "v", "t3": "g", "pn": "v",
    "fc": "s", "fs": "s",
    "tp0": "s", "tp1": "g",  # transpose piece engine, alternating
    # chunk sub-ops
    "sT": "s", "cT": "s", "s2": "s", "c2": "s", "lg": "s", "rv": "s",
    "ss": "g", "cdm": "g", "sdm": "v",
}
ENG.update(json.loads(os.environ.get("KERN_ENG", "{}")))
DT32 = os.environ.get("DT32", "0") == "1"
DT = F32 if DT32 else F16
NPDT = np.float32 if DT32 else np.float16


def _emit(nc: bass.Bass):
    angles = nc.dram_tensor("angles", [BS, 2 * N], DT, kind="ExternalInput").ap()
    prev = nc.dram_tensor("prev_three", [BS, 3, 3], F32, kind="ExternalInput").ap()
    out = nc.dram_tensor("out", [BS, N, 3], DT, kind="ExternalOutput").ap()

    ang_flat = angles.rearrange("(p j) c -> p (j c)", p=P)      # [128, 16128]
    prev_flat = prev.rearrange("(p j) r c -> p (j r c)", p=P)   # [128, 576]
    out_flat = out.rearrange("(p j) a c -> p (j a c)", p=P)     # [128, 24192]

    def eng(key):
        return {"v": nc.vector, "g": nc.gpsimd, "s": nc.scalar}[ENG[key]]

    with tile.TileContext(nc) as tc:
        with (
            tc.tile_pool(name="raw", bufs=1) as rawp,
            tc.tile_pool(name="planes", bufs=1) as planesp,
            tc.tile_pool(name="chunk", bufs=1) as chunkp,
            tc.tile_pool(name="state", bufs=2) as statep,
            tc.tile_pool(name="scratch", bufs=2) as scratch,
            tc.tile_pool(name="big", bufs=1) as bigp,
            tc.tile_pool(name="pv", bufs=1) as pvp,
        ):
            raw = rawp.tile([P, J * 2 * N], DT, tag="raw")       # j-major
            cdp = planesp.tile([P, N * J], DT, tag="cdp")        # atom-major
            sdp = planesp.tile([P, N * J], DT, tag="sdp")
            # ring of 2*TP atoms of dense p; transpose pieces drain it
            pblock = bigp.tile([P, 2 * TP * 3 * J], DT, tag="pblock")
            stag = bigp.tile([P, J * N * 3], DT, tag="stag")      # j-major p
            pv = pvp.tile([P, J * 9], F32, tag="pv")
            pvt = pvp.tile([P, 9 * J], F32, tag="pvt")
            ln24 = pvp.tile([P, 1], F32, tag="ln24")
            nc.vector.memset(ln24[:], float(np.log(24.0)))

            nc.sync.dma_start(out=pv[:], in_=prev_flat)
            nc.sync.dma_start(out=raw[:], in_=ang_flat)

            raw_aj = raw[:].rearrange("p (j a) -> p a j", a=2 * N)  # strided view
            cd_a = cdp[:].rearrange("p (a j) -> p a j", j=J)
            sd_a = sdp[:].rearrange("p (a j) -> p a j", j=J)
            pb_v = pblock[:].rearrange("p (a x) -> p a x", x=3 * J)  # per-atom dense
            stag_r = stag[:].rearrange("p (j x) -> p x j", x=3 * N)

            # ---------- precompute: one chunk pair, phase-batched --------
            def pair_ops(k0):
                """Thunks for chunks k0, k0+1 (atoms [CH*k0, CH*k0+2*CH))."""
                ops = []
                tiles = {}
                for ci, k in enumerate((k0, k0 + 1)):
                    if k * CH >= N:
                        continue
                    a0 = k * CH
                    sl = slice(a0, a0 + CH)
                    csl = slice(N + a0, N + a0 + CH)
                    sfx = "a" if ci == 0 else "b"
                    SH = [P, CH, J]
                    st = chunkp.tile(SH, DT, tag=f"sT{sfx}", name=f"sT{k}")
                    ct = chunkp.tile(SH, DT, tag=f"cT{sfx}", name=f"cT{k}")
                    s2 = chunkp.tile(SH, DT, tag=f"s2{sfx}", name=f"s2{k}")
                    c2 = chunkp.tile(SH, DT, tag=f"c2{sfx}", name=f"c2{k}")
                    ssq = chunkp.tile(SH, DT, tag=f"ss{sfx}", name=f"ss{k}")
                    lg = chunkp.tile(SH, DT, tag=f"s2{sfx}", name=f"lg{k}")
                    rv = chunkp.tile(SH, DT, tag=f"c2{sfx}", name=f"rv{k}")
                    tiles[k] = (st, ct, s2, c2, ssq, lg, rv, sl, csl)
                # phase-interleaved order: all sT, all cT, ..., so Ln/Exp batch
                for phase in range(9):
                    for k in (k0, k0 + 1):
                        if k not in tiles:
                            continue
                        st, ct, s2, c2, ssq, lg, rv, sl, csl = tiles[k]
                        # fp16-safe rsqrt: (24s)^2+(24c)^2 keeps tiny dihedrals
                        # out of fp16-subnormal range; Exp bias ln(24) undoes
                        # the scale exactly: exp(-0.5*ln(576*ss)+ln24) = ss^-0.5
                        if phase == 0:
                            ops.append(lambda st=st, sl=sl:
                                       eng("sT").copy(st[:], raw_aj[:, sl, :]))
                        elif phase == 1:
                            ops.append(lambda ct=ct, csl=csl:
                                       eng("cT").copy(ct[:], raw_aj[:, csl, :]))
                        elif phase == 2:
                            ops.append(lambda s2=s2, st=st:
                                       eng("s2").activation(s2[:], st[:], AF.Square, 0.0, 24.0))
                        elif phase == 3:
                            ops.append(lambda c2=c2, ct=ct:
                                       eng("c2").activation(c2[:], ct[:], AF.Square, 0.0, 24.0))
                        elif phase == 4:
                            ops.append(lambda ssq=ssq, s2=s2, c2=c2:
                                       eng("ss").tensor_add(ssq[:], s2[:], c2[:]))
                        elif phase == 5:
                            ops.append(lambda lg=lg, ssq=ssq:
                                       eng("lg").activation(lg[:], ssq[:], AF.Ln))
                        elif phase == 6:
                            ops.append(lambda rv=rv, lg=lg:
                                       eng("rv").activation(
                                           rv[:], lg[:], AF.Exp, ln24[:], -0.5))
                        elif phase == 7:
                            ops.append(lambda ct=ct, rv=rv, sl=sl:
                                       eng("cdm").tensor_mul(cd_a[:, sl, :], ct[:], rv[:]))
                        else:
                            ops.append(lambda st=st, rv=rv, sl=sl:
                                       eng("sdm").tensor_mul(sd_a[:, sl, :], st[:], rv[:]))
                return ops

            # ---------- initial frame from prev_three (fp32) -------------
            nc.scalar.copy(
                pvt[:].rearrange("p (x j) -> p x j", x=9),
                pv[:].rearrange("p (j x) -> p x j", x=9),
            )
            pvt_r = pvt[:].rearrange("p (x j) -> p x j", x=9)
            a0_ap = pvt_r[:, 0:3, :]
            b0_ap = pvt_r[:, 3:6, :]
            c0_ap = pvt_r[:, 6:9, :]

            def rsqrt3(dst, src3, tag):
                sq = scratch.tile([P, 3, J], F32, tag="i_sq", name=f"sq_{tag}")
                nc.scalar.square(sq[:], src3[:])
                s1 = scratch.tile([P, 1, J], F32, tag="i_s1", name=f"s1_{tag}")
                nc.vector.tensor_add(s1[:], sq[:, 0:1, :], sq[:, 1:2, :])
                s2_ = scratch.tile([P, 1, J], F32, tag="i_s2", name=f"s2_{tag}")
                nc.vector.tensor_add(s2_[:], s1[:], sq[:, 2:3, :])
                lgi = scratch.tile([P, 1, J], F32, tag="i_lg", name=f"lg_{tag}")
                nc.scalar.activation(lgi[:], s2_[:], AF.Ln)
                nc.scalar.activation(dst[:], lgi[:], AF.Exp, 0.0, -0.5)

            def cross_into(dst, x, y, eps):
                # dst[c] = (x[c1]*y[c2] + eps) - x[c2]*y[c1]
                for c in range(3):
                    c1, c2 = (c + 1) % 3, (c + 2) % 3
                    m = scratch.tile([P, 1, J], F32, tag="i_cm", name=f"cm{c}_{dst.name}")
                    q = scratch.tile([P, 1, J], F32, tag="i_cq", name=f"cq{c}_{dst.name}")
                    nc.vector.tensor_mul(m[:], x[:, c1 : c1 + 1, :], y[:, c2 : c2 + 1, :])
                    nc.vector.tensor_mul(q[:], x[:, c2 : c2 + 1, :], y[:, c1 : c1 + 1, :])
                    nc.vector.scalar_tensor_tensor(
                        dst[:, c : c + 1, :], m[:], eps, q[:], AOP.add, AOP.subtract)

            vv = scratch.tile([P, 3, J], F32, tag="i_vv")
            nc.vector.scalar_tensor_tensor(
                vv[:], b0_ap, 1e-8, c0_ap, AOP.add, AOP.subtract)
            rv1 = scratch.tile([P, 1, J], F32, tag="i_rv")
            rsqrt3(rv1, vv, "f1")
            f1f = scratch.tile([P, 3, J], F32, tag="i_f1")
            nc.vector.tensor_mul(f1f[:], vv[:], rv1[:].broadcast_to([P, 3, J]))
            g1 = statep.tile([P, 3, J], DT, tag="g1", name="g1_init")
            nc.scalar.copy(g1[:], f1f[:])

            uu = scratch.tile([P, 3, J], F32, tag="i_uu")
            nc.vector.tensor_sub(uu[:], b0_ap, a0_ap)
            # m3_init = +f3 = normalize(cross(b-a, f1) + 1e-8)
            ww = scratch.tile([P, 3, J], F32, tag="i_ww", name="i_ww")
            cross_into(ww, uu, f1f, 1e-8)
            rv2 = scratch.tile([P, 1, J], F32, tag="i_rv", name="i_rv2")
            rsqrt3(rv2, ww, "f3")
            f3f = scratch.tile([P, 3, J], F32, tag="i_f3")
            nc.vector.tensor_mul(f3f[:], ww[:], rv2[:].broadcast_to([P, 3, J]))
            m3 = statep.tile([P, 3, J], DT, tag="m3", name="m3_init")
            nc.scalar.copy(m3[:], f3f[:])
            # g2h_init = -f2 = cross(f1, f3)
            g2f = scratch.tile([P, 3, J], F32, tag="i_g2", name="i_g2")
            cross_into(g2f, f1f, f3f, 0.0)
            g2h = statep.tile([P, 3, J], DT, tag="g2", name="g2_init")
            nc.scalar.copy(g2h[:], g2f[:])
            # p_init = c0 in DT
            p0 = statep.tile([P, 3, J], DT, tag="pd", name="p_init")
            nc.scalar.copy(p0[:], c0_ap)
            p_prev_ap = p0[:]

            # pre-emit first chunk pair; pace the rest at 1 op / 2 atoms
            for f in pair_ops(0):
                f()
            pending = pair_ops(2)
            next_pair = 4

            # ---------- main recurrence ---------------------------------
            for i in range(N):
                k3 = i % 3
                ca, sa = float(_CA[k3]), float(_SA[k3])
                sbond = float(_BOND[k3] * (1.0 if i % 2 == 0 else -1.0))

                cdb = cd_a[:, i : i + 1, :].broadcast_to([P, 3, J])
                sdb = sd_a[:, i : i + 1, :].broadcast_to([P, 3, J])

                fc = scratch.tile([P, 3, J], DT, tag="fc", name=f"fc{i}")
                eng("fc").mul(fc[:], g1[:], ca)
                fs = scratch.tile([P, 3, J], DT, tag="fs", name=f"fs{i}")
                eng("fs").mul(fs[:], g1[:], sa)

                t3 = scratch.tile([P, 3, J], DT, tag="t3", name=f"t3_{i}")
                eng("t3").tensor_mul(t3[:], g2h[:], sdb)
                t1 = scratch.tile([P, 3, J], DT, tag="t1", name=f"t1_{i}")
                eng("t1").tensor_mul(t1[:], g2h[:], cdb)
                t2 = scratch.tile([P, 3, J], DT, tag="t2", name=f"t2_{i}")
                eng("t2").tensor_mul(t2[:], m3[:], sdb)
                t4 = scratch.tile([P, 3, J], DT, tag="t4", name=f"t4_{i}")
                eng("t4").tensor_mul(t4[:], m3[:], cdb)
                hh = scratch.tile([P, 3, J], DT, tag="hh", name=f"hh{i}")
                eng("hh").tensor_add(hh[:], t1[:], t2[:])

                g1n = statep.tile([P, 3, J], DT, tag="g1", name=f"g1_{i}")
                eng("g1n").scalar_tensor_tensor(
                    g1n[:], hh[:], -sa, fc[:], AOP.mult, AOP.add)
                g2n = statep.tile([P, 3, J], DT, tag="g2", name=f"g2_{i}")
                eng("g2n").scalar_tensor_tensor(
                    g2n[:], hh[:], ca, fs[:], AOP.mult, AOP.add)
                m3n = statep.tile([P, 3, J], DT, tag="m3", name=f"m3_{i}")
                eng("m3n").tensor_sub(m3n[:], t3[:], t4[:])

                slot = i % (2 * TP)
                pn_ap = pblock[:][:, 3 * J * slot : 3 * J * (slot + 1)].rearrange(
                    "p (c j) -> p c j", c=3)
                eng("pn").scalar_tensor_tensor(
                    pn_ap, g1n[:], sbond, p_prev_ap, AOP.mult, AOP.add)
                p_prev_ap = pn_ap
                g1, g2h, m3 = g1n, g2n, m3n

                # pace precompute 1 op / 2 atoms
                if i % 2 == 0 and pending:
                    pending.pop(0)()
                elif i % 2 == 0 and next_pair * CH < N:
                    pending = pair_ops(next_pair)
                    next_pair += 2
                    pending.pop(0)()

                # transpose piece each TP atoms: atoms [i-TP+1 .. i]
                # dims [P, j, x]: contiguous 36B writes, gathered reads
                if i % TP == TP - 1:
                    pc = i // TP
                    rsl = slice(3 * TP * (pc % 2), 3 * TP * (pc % 2 + 1))
                    xsl = slice(3 * TP * pc, 3 * TP * (pc + 1))
                    src = pblock[:].rearrange(
                        "p (x j) -> p j x", j=J)[:, :, rsl]
                    dst = stag[:].rearrange(
                        "p (j x) -> p j x", x=3 * N)[:, :, xsl]
                    e = eng("tp0") if pc % 2 == 0 else eng("tp1")
                    if e is nc.scalar:
                        e.copy(dst, src)
                    else:
                        e.tensor_copy(dst, src)

            nc.sync.dma_start(out=out_flat, in_=stag[:])
    return nc


_NC_CACHE: dict = {}


def _get_nc():
    if "nc" not in _NC_CACHE:
        nc = bacc.Bacc("TRN2", target_bir_lowering=False, debug=False)
        _emit(nc)
        nc.compile()
        _NC_CACHE["nc"] = nc
    return _NC_CACHE["nc"]


def run_sharded(angles: np.ndarray, prev_three: np.ndarray, **kw):
    angles = np.ascontiguousarray(angles).astype(NPDT)
    prev_three = np.ascontiguousarray(prev_three, np.float32)
    assert angles.shape == (B_FULL, 2 * N) and prev_three.shape == (B_FULL, 3, 3)
    in_maps = [
        {
            "angles": angles[i * BS : (i + 1) * BS],
            "prev_three": prev_three[i * BS : (i + 1) * BS],
        }
        for i in range(N_CORES)
    ]
    return run_bass_kernel_spmd(_get_nc(), in_maps, core_ids=list(range(N_CORES)), **kw)


def kernel(angles: np.ndarray, prev_three: np.ndarray) -> np.ndarray:
    res = run_sharded(angles, prev_three)
    return np.concatenate(
        [r["out"].astype(np.float32) for r in res.results], axis=0)
